# revision 1
# baseline (speedup 1.0000x reference)
"""DBRX attention block (B=1, T=2048, D=6144, 48 q heads / 8 kv heads, RoPE,
clamp, causal) as a Bass/Tile kernel on 8 Trainium2 NeuronCores.

Sharding: tensor-parallel over heads. Core c owns q heads [6c, 6c+6) and kv
head c (GQA groups align exactly: q head i uses kv head i//6). Each core
computes its slice of the QKV projection from the full x, runs RoPE + clamp +
causal attention for its 6 heads, then its partial output projection
(w_out row-sharded); the 8 partial outputs are summed on the host.

On-chip layout is feature-major ([feature, T]) throughout, so every matmul
is a clean [128 x 128] x [128 x 512] fp32r PE op at full rate. Matmuls are
scheduled weight-stationary where possible (one lhsT reused across 4 PSUM
banks) to amortize the 4-byte LDWEIGHTS:
  - qkvT[f, t]  = sum_d wqkvT[d, f] * xT[d, t]       (PSUM-accumulated over d)
  - RoPE via a pair-swap permutation matmul + DVE combine with cos/sin tables
  - scoresT[k, q] = sum_h kT[h, k] * qT[h, q], exp on ACT (no max-subtraction:
    scores are bounded by clamp at +-8, empirically |s| < ~25, exp fits fp32)
  - attnT[h, q] = sum_k v[k, h] * expT[k, q]; softmax denominator from a
    ones-vector matmul, applied via reciprocal + DMA partition-broadcast
  - outT[o, t] = sum_f w_o[f, o] * attnT[f, t]
"""

import math
import sys
from contextlib import ExitStack

import numpy as np

for _p in ("/opt/trn_rl_repo", "/root/.axon_site/_ro/trn_rl_repo"):
    if _p not in sys.path:
        sys.path.append(_p)

import concourse.bass as bass
import concourse.tile as tile
from concourse import bacc, mybir
from concourse.bass_utils import run_bass_kernel_spmd

F32 = mybir.dt.float32
F32R = mybir.dt.float32r
MULT = mybir.AluOpType.mult
ADD = mybir.AluOpType.add
MAX = mybir.AluOpType.max
MIN = mybir.AluOpType.min
EXP = mybir.ActivationFunctionType.Exp

N_CORES = 8
D_MODEL = 6144
N_HEADS = 48
N_KV_HEADS = 8
HEAD_DIM = 128
NQH = N_HEADS // N_CORES          # q heads per core = 6
T = 2048
CLAMP = 8.0
ROPE_BASE = 500000.0
SCALE = 1.0 / math.sqrt(HEAD_DIM)


def _chunk_plan(kd, chunk):
    """Chunk sizes: small leading chunks so the PE starts early."""
    plan = []
    rest = kd
    if kd > 8:
        plan = [1, 2, 3]
        rest = kd - 6
    while rest > 0:
        c = min(chunk, rest)
        plan.append(c)
        rest -= c
    return plan


def build_program(d_model=D_MODEL, t=T, nqh=NQH, chunk=6, reps=1, stop_after=None):
    """Build the per-core Bass program. Returns the compiled Bacc handle."""
    kd = d_model // 128            # contraction tiles for qkv projection
    tq = t // 512                  # 512-wide T quads
    tb = t // 128                  # 128-wide T blocks
    nf = nqh + 2                   # feature tiles: q heads + k + v
    qkv_cols = nf * 128

    nc = bacc.Bacc("TRN2", target_bir_lowering=False, debug=False)

    xT = nc.dram_tensor("xT", [d_model, t], F32, kind="ExternalInput").ap()
    wqkvT = nc.dram_tensor("wqkvT", [d_model, qkv_cols], F32, kind="ExternalInput").ap()
    w_o = nc.dram_tensor("w_o", [nqh * 128, d_model], F32, kind="ExternalInput").ap()
    cosf = nc.dram_tensor("cosf", [128, t], F32, kind="ExternalInput").ap()
    sinf = nc.dram_tensor("sinf", [128, t], F32, kind="ExternalInput").ap()
    maskT = nc.dram_tensor("maskT", [4, 128, 512], F32, kind="ExternalInput").ap()
    pswap = nc.dram_tensor("pswap", [128, 128], F32, kind="ExternalInput").ap()
    ident = nc.dram_tensor("ident", [128, 128], F32, kind="ExternalInput").ap()
    ones = nc.dram_tensor("ones", [128, 1], F32, kind="ExternalInput").ap()
    outT = nc.dram_tensor("outT", [d_model, t], F32, kind="ExternalOutput").ap()

    def _body(tc):
        with ExitStack() as octx:
            consts = octx.enter_context(tc.tile_pool(name="consts", bufs=1))
            accp = octx.enter_context(tc.tile_pool(name="accp", bufs=nf))

            sb_cos = consts.tile([128, t], F32, tag="cos")
            nc.sync.dma_start(sb_cos[:], cosf[:])
            sb_sin = consts.tile([128, t], F32, tag="sin")
            nc.sync.dma_start(sb_sin[:], sinf[:])
            sb_mask = consts.tile([128, 4, 512], F32, tag="mask")
            nc.sync.dma_start(sb_mask[:], maskT.transpose([1, 0, 2]))
            sb_pswap = consts.tile([128, 128], F32R, tag="pswap")
            nc.gpsimd.dma_start(sb_pswap[:], pswap[:])
            sb_ident = consts.tile([128, 128], F32R, tag="ident")
            nc.gpsimd.dma_start(sb_ident[:], ident[:])
            sb_ones = consts.tile([128, 1], F32R, tag="ones")
            nc.gpsimd.dma_start(sb_ones[:], ones[:])

            acc = [accp.tile([128, t], F32R, tag="acc", name=f"acc{i}")
                   for i in range(nf)]

            rep_ctx = ExitStack()
            if reps > 1:
                rep_ctx.enter_context(tc.For_i(0, reps, 1))
            octx.enter_context(rep_ctx)

            # ---------------- Phase 1: qkvT = wqkvT-tiles.T @ xT ----------------
            # Weight-stationary: one w block feeds all t-quads (4 PSUM banks).
            with ExitStack() as ctx:
                xp = ctx.enter_context(tc.tile_pool(name="xp", bufs=10))
                wp = ctx.enter_context(tc.tile_pool(name="wp", bufs=9))
                psp = ctx.enter_context(tc.tile_pool(name="psp", bufs=2 * tq,
                                                     space="PSUM"))
                plan = _chunk_plan(kd, chunk)
                base = 0
                for c, csz in enumerate(plan):
                    ks = list(range(base, base + csz))
                    base += csz
                    xts, wts = [], []
                    for k in ks:
                        xt = xp.tile([128, t], F32R, tag="xt")
                        nc.gpsimd.dma_start(xt[:], xT[k * 128:(k + 1) * 128, :])
                        xts.append(xt)
                        wt = wp.tile([128, qkv_cols], F32R, tag="wt")
                        nc.gpsimd.dma_start(wt[:], wqkvT[k * 128:(k + 1) * 128, :])
                        wts.append(wt)
                    for f in range(nf):
                        pss = [psp.tile([128, 512], F32, tag="ps", name=f"ps{q}")
                               for q in range(tq)]
                        for i in range(len(ks)):
                            lhs = wts[i][:, f * 128:(f + 1) * 128]
                            for q in range(tq):
                                nc.tensor.matmul(
                                    pss[q][:], lhs, xts[i][:, q * 512:(q + 1) * 512],
                                    start=(i == 0), stop=(i == len(ks) - 1),
                                )
                        for q in range(tq):
                            sl = slice(q * 512, (q + 1) * 512)
                            if c == 0:
                                nc.vector.tensor_copy(acc[f][:, sl], pss[q][:])
                            else:
                                nc.vector.scalar_tensor_tensor(
                                    out=acc[f][:, sl], in0=pss[q][:], scalar=1.0,
                                    in1=acc[f][:, sl], op0=MULT, op1=ADD,
                                )

            if stop_after == "p1":
                ob0 = consts.tile([128, 512], F32, tag="dbg")
                nc.vector.tensor_copy(ob0[:], acc[0][:, 0:512].bitcast(F32))
                nc.sync.dma_start(outT[0:128, 0:512], ob0[:])

            if stop_after == "p1":
                return
            # late pools: open only after phase 1 frees its SBUF
            attp = octx.enter_context(tc.tile_pool(name="attp", bufs=nqh))
            vtp = octx.enter_context(tc.tile_pool(name="vtp", bufs=1))
            attnT = [attp.tile([128, t], F32R, tag="attnT", name=f"attnT{i}")
                     for i in range(nqh)]
            v_t = vtp.tile([128, tb, 128], F32R, tag="v_t")

            # ---------------- Phase 1.5: RoPE + clamp + v transpose ----------------
            # Emission order matters (DVE is FIFO): k head first, then v work,
            # then q heads in pair order — the attention loop below is
            # hp-outer, so rope for later head pairs hides under the PE work
            # of earlier pairs.
            rope_ctx = ExitStack()
            # shared PSUM pool: rope pair-swap tiles + attention score tiles
            spp = rope_ctx.enter_context(tc.tile_pool(name="spp", bufs=4,
                                                      space="PSUM"))
            tmp = rope_ctx.enter_context(tc.tile_pool(name="rtmp", bufs=3))

            def emit_rope(f):
                for q in range(tq):
                    sl = slice(q * 512, (q + 1) * 512)
                    pshuf = spp.tile([128, 512], F32, tag="ps", name="pshuf")
                    nc.tensor.matmul(pshuf[:], sb_pswap[:], acc[f][:, sl],
                                     start=True, stop=True)
                    t1 = tmp.tile([128, 512], F32, tag="t1", name="t1")
                    nc.vector.tensor_tensor(t1[:], acc[f][:, sl].bitcast(F32),
                                            sb_cos[:, sl], op=MULT)
                    t2 = tmp.tile([128, 512], F32, tag="t2", name="t2")
                    nc.vector.tensor_tensor(t2[:], pshuf[:], sb_sin[:, sl], op=MULT)
                    t3 = tmp.tile([128, 512], F32, tag="t3", name="t3")
                    nc.vector.tensor_tensor(t3[:], t1[:], t2[:], op=ADD)
                    nc.vector.tensor_scalar(acc[f][:, sl], t3[:], -CLAMP, CLAMP,
                                            op0=MAX, op1=MIN)

            def emit_vwork():
                iv = nqh + 1                      # v: clamp only, then transpose
                for q in range(tq):
                    sl = slice(q * 512, (q + 1) * 512)
                    t4 = tmp.tile([128, 512], F32, tag="t1", name="t1")
                    nc.vector.tensor_scalar(t4[:], acc[iv][:, sl].bitcast(F32),
                                            -CLAMP, CLAMP, op0=MAX, op1=MIN)
                    nc.vector.tensor_copy(acc[iv][:, sl], t4[:])
                for j in range(tb):
                    pt = spp.tile([128, 128], F32R, tag="ps", name="pt")
                    nc.tensor.transpose(pt[:], acc[iv][:, j * 128:(j + 1) * 128],
                                        sb_ident[:])
                    nc.vector.tensor_copy(v_t[:, j, :], pt[:])

            emit_rope(nqh)                        # k head
            emit_vwork()

            if stop_after == "rope":
                emit_rope(0)
                for f in range(1, nqh):
                    emit_rope(f)
                ob0 = consts.tile([128, 512], F32, tag="dbg")
                nc.vector.tensor_copy(ob0[:, 0:128], v_t[:, 0, :].bitcast(F32))
                nc.vector.tensor_copy(ob0[:, 128:256], acc[0][:, 0:128].bitcast(F32))
                nc.sync.dma_start(outT[0:128, 0:512], ob0[:])
                rope_ctx.close()
                return
            # ---------------- Phase 2: causal attention (head pairs) ----------------
            # Software-pipelined: score matmuls + exp run PIPE kb-steps ahead
            # of the PV/denominator matmuls so the PE never waits on the ACT
            # round-trip. PSUM: 4 score banks + 2 attn banks + 2 sum banks.
            # hp-outer so emit_rope(f) for pair p+1 overlaps pair p's matmuls.
            PIPE = 2
            with ExitStack() as ctx:
                exps = ctx.enter_context(tc.tile_pool(name="exps", bufs=2 * (PIPE + 1)))
                psa_p = ctx.enter_context(tc.tile_pool(name="psa", bufs=2, space="PSUM"))
                psm_p = ctx.enter_context(tc.tile_pool(name="psm", bufs=2, space="PSUM"))
                rcp = ctx.enter_context(tc.tile_pool(name="rcp", bufs=2))
                rcb = ctx.enter_context(tc.tile_pool(name="rcb", bufs=2))
                ik = nqh                          # k head feature tile
                for hp in range(nqh // 2):
                    hs = (2 * hp, 2 * hp + 1)
                    emit_rope(hs[0])
                    emit_rope(hs[1])
                    for J in range(tq):
                        qsl = slice(J * 512, (J + 1) * 512)
                        nkb = 4 * J + 4
                        pa = {h: psa_p.tile([128, 512], F32, tag="pa", name="pa")
                              for h in hs}
                        pm = {h: psm_p.tile([1, 512], F32, tag="pm", name="pm")
                              for h in hs}
                        exq = {}
                        for kb in range(nkb + PIPE):
                            if kb < nkb:
                                klhs = acc[ik][:, kb * 128:(kb + 1) * 128]
                                for h in hs:
                                    ps = spp.tile([128, 512], F32, tag="ps",
                                                  name="ps")
                                    nc.tensor.matmul(ps[:], klhs, acc[h][:, qsl],
                                                     start=True, stop=True)
                                    if kb >= 4 * J:
                                        nc.vector.tensor_tensor(
                                            ps[:], ps[:],
                                            sb_mask[:, kb - 4 * J, :], op=ADD)
                                    ex = exps.tile([128, 512], F32R, tag="ex",
                                                   name="ex")
                                    nc.scalar.activation(ex[:], ps[:], EXP,
                                                         scale=SCALE)
                                    exq[(kb, h)] = ex
                            kbd = kb - PIPE
                            if kbd < 0 or kbd >= nkb:
                                continue
                            st = (kbd == 0)
                            sp = (kbd == nkb - 1)
                            for h in hs:
                                nc.tensor.matmul(pa[h][:], v_t[:, kbd, :],
                                                 exq[(kbd, h)][:], start=st, stop=sp)
                            for h in hs:
                                nc.tensor.matmul(pm[h][:], sb_ones[:],
                                                 exq.pop((kbd, h))[:],
                                                 start=st, stop=sp)
                        for h in hs:
                            rc = rcp.tile([1, 512], F32R, tag="rc", name="rc")
                            nc.vector.reciprocal(rc[:], pm[h][:])
                            rb = rcb.tile([128, 512], F32R, tag="rb", name="rb")
                            nc.sync.dma_start(
                                out=rb[:],
                                in_=bass.AP(tensor=rc.tensor, offset=rc.offset,
                                            ap=[[1, 1], [0, 128]] + rc.ap[1:]))
                            nc.vector.tensor_tensor(attnT[h][:, qsl], pa[h][:],
                                                    rb[:], op=MULT)

            rope_ctx.close()

            if stop_after == "attn":
                ob0 = consts.tile([128, 512], F32, tag="dbg")
                nc.vector.tensor_copy(ob0[:], attnT[0][:, 0:512].bitcast(F32))
                nc.sync.dma_start(outT[0:128, 0:512], ob0[:])
                return
            # ---------------- Phase 3: partial out projection ----------------
            # Weight-stationary: one w_o block feeds all 4 t-quads.
            with ExitStack() as ctx:
                wop = ctx.enter_context(tc.tile_pool(name="wop", bufs=4))
                outp = ctx.enter_context(tc.tile_pool(name="outp", bufs=2 * tq))
                pso = ctx.enter_context(tc.tile_pool(name="pso", bufs=2 * tq,
                                                     space="PSUM"))
                for o in range(d_model // 128):
                    wo = wop.tile([128, nqh, 128], F32R, tag="wo")
                    src = w_o[:, o * 128:(o + 1) * 128].rearrange(
                        "(s p) o -> p s o", p=128)
                    nc.gpsimd.dma_start(wo[:], src)
                    pos = [pso.tile([128, 512], F32, tag="po", name=f"po{q}")
                           for q in range(tq)]
                    for s in range(nqh):
                        lhs = wo[:, s, :]
                        for J in range(tq):
                            nc.tensor.matmul(pos[J][:], lhs,
                                             attnT[s][:, J * 512:(J + 1) * 512],
                                             start=(s == 0), stop=(s == nqh - 1))
                    for J in range(tq):
                        ob = outp.tile([128, 512], F32, tag="ob", name="ob")
                        nc.vector.tensor_copy(ob[:], pos[J][:])
                        nc.sync.dma_start(
                            outT[o * 128:(o + 1) * 128, J * 512:(J + 1) * 512], ob[:])

    with tile.TileContext(nc) as tc, nc.allow_low_precision(reason="fp32r matmuls"):
        _body(tc)
    nc.compile()
    return nc


def make_core_inputs(x, causal_mask, w_qkv, w_out, d_model=D_MODEL, t=T, nqh=NQH,
                     n_cores=N_CORES):
    """Host-side sharding: per-core input dicts for the SPMD program."""
    x2 = np.ascontiguousarray(x.reshape(t, d_model).T)      # [D, T]
    kv_base = nqh * n_cores * HEAD_DIM
    n_kv = n_cores

    # RoPE tables in [head_dim, T] layout (interleaved-pair convention).
    hd = HEAD_DIM
    inv = 1.0 / ROPE_BASE ** (np.arange(0, hd, 2, dtype=np.float64) / hd)
    pos = np.arange(t, dtype=np.float64)
    freqs = pos[None, :] * inv[:, None]                     # [hd/2, T]
    cos = np.cos(freqs)
    sin = np.sin(freqs)
    cosf = np.empty((hd, t), dtype=np.float32)
    sinf = np.empty((hd, t), dtype=np.float32)
    cosf[0::2] = cos
    cosf[1::2] = cos
    sinf[0::2] = -sin                                       # row 2i:   -sin
    sinf[1::2] = sin                                        # row 2i+1: +sin

    cm = causal_mask.reshape(causal_mask.shape[-2], causal_mask.shape[-1])
    maskT = np.stack([np.ascontiguousarray(cm[0:512, m * 128:(m + 1) * 128].T)
                      for m in range(4)]).astype(np.float32)

    pswap = np.zeros((128, 128), dtype=np.float32)
    for i in range(0, 128, 2):
        pswap[i, i + 1] = 1.0
        pswap[i + 1, i] = 1.0
    ident = np.eye(128, dtype=np.float32)

    in_maps = []
    for c in range(n_cores):
        qrows = np.arange(c * nqh * 128, (c + 1) * nqh * 128)
        krows = np.arange(kv_base + c * 128, kv_base + (c + 1) * 128)
        vrows = np.arange(kv_base + n_kv * 128 + c * 128,
                          kv_base + n_kv * 128 + (c + 1) * 128)
        rows = np.concatenate([qrows, krows, vrows])
        wqkvT_c = np.ascontiguousarray(w_qkv[rows, :].T)    # [D, nqh*128+256]
        w_o_c = np.ascontiguousarray(w_out[:, qrows].T)     # [nqh*128, D]
        in_maps.append({
            "xT": x2, "wqkvT": wqkvT_c, "w_o": w_o_c,
            "cosf": cosf, "sinf": sinf, "maskT": maskT,
            "pswap": pswap, "ident": ident,
            "ones": np.ones((128, 1), dtype=np.float32),
        })
    return in_maps


_PROGRAM_CACHE = {}


def _get_program():
    key = (D_MODEL, T, NQH)
    if key not in _PROGRAM_CACHE:
        _PROGRAM_CACHE[key] = build_program()
    return _PROGRAM_CACHE[key]


def kernel(x, causal_mask, w_qkv, w_out):
    x = np.asarray(x, dtype=np.float32)
    causal_mask = np.asarray(causal_mask, dtype=np.float32)
    w_qkv = np.asarray(w_qkv, dtype=np.float32)
    w_out = np.asarray(w_out, dtype=np.float32)

    nc = _get_program()
    in_maps = make_core_inputs(x, causal_mask, w_qkv, w_out)
    res = run_bass_kernel_spmd(nc, in_maps, list(range(N_CORES)))
    outT = np.zeros((D_MODEL, T), dtype=np.float32)
    for c in range(N_CORES):
        outT += res.results[c]["outT"]
    return np.ascontiguousarray(outT.T).reshape(1, T, D_MODEL).astype(np.float32)



# revision 9
# speedup vs baseline: 2.8415x; 2.8415x over previous
"""DBRX attention block (B=1, T=2048, D=6144, 48 q heads / 8 kv heads, RoPE,
clamp, causal) as a Bass/Tile kernel on 8 Trainium2 NeuronCores.

Sharding: tensor-parallel over heads. Core c owns q heads [6c, 6c+6) and kv
head c (GQA groups align exactly: q head i uses kv head i//6).

Wire-traffic-optimized layout (the axon tunnel re-ships every operand on
each execution, ~10 GB/s aggregate, so operand bytes dominate wall time):
  - all inputs are shipped in fp16 (w_qkv / w_out / x / tables),
  - x is shipped SHARDED over the 8 cores (768 rows of xT each) and
    all-gathered on-device over NeuronLink (saves 7x duplication),
  - each core's partial output projection is reduce-scattered on-device, so
    each core ships back only its [768, 2048] fp16 shard of the final output
    (instead of a full [6144, 2048] fp32 partial per core).

On-chip layout is feature-major ([feature, T]) throughout, so every matmul
is a clean [128 x 128] x [128 x 512] PE op at full rate. The QKV projection
and the output projection run in fp16 (inputs are fp16; PSUM accumulates
fp32). The attention core (RoPE combine, exp, PV) stays fp32r because the
no-max-subtraction exp needs fp32 range. Matmuls are scheduled
weight-stationary (one lhsT reused across 4 PSUM banks):
  - qkvT[f, t]  = sum_d wqkvT[d, f] * xT[d, t]       (PSUM-accumulated over d)
  - RoPE via a pair-swap permutation matmul + DVE combine with cos/sin tables
  - scoresT[k, q] = sum_h kT[h, k] * qT[h, q], exp on ACT (no max-subtraction:
    scores are bounded by clamp at +-8, empirically |s| < ~25, exp fits fp32)
  - attnT[h, q] = sum_k v[k, h] * expT[k, q]; softmax denominator from a
    ones-vector matmul, applied via reciprocal + DMA partition-broadcast
  - outT[o, t] = sum_f w_o[f, o] * attnT[f, t]        (fp16)
"""

import math
import sys
from contextlib import ExitStack

import numpy as np

for _p in ("/opt/trn_rl_repo", "/root/.axon_site/_ro/trn_rl_repo"):
    if _p not in sys.path:
        sys.path.append(_p)

import concourse.bass as bass
import concourse.tile as tile
from concourse import bacc, mybir
from concourse.bass_utils import run_bass_kernel_spmd

F16 = mybir.dt.float16
F32 = mybir.dt.float32
F32R = mybir.dt.float32r
MULT = mybir.AluOpType.mult
ADD = mybir.AluOpType.add
MAX = mybir.AluOpType.max
MIN = mybir.AluOpType.min
EXP = mybir.ActivationFunctionType.Exp

N_CORES = 8
D_MODEL = 6144
N_HEADS = 48
N_KV_HEADS = 8
HEAD_DIM = 128
NQH = N_HEADS // N_CORES          # q heads per core = 6
T = 2048
CLAMP = 8.0
ROPE_BASE = 500000.0
SCALE = 1.0 / math.sqrt(HEAD_DIM)
MASK_NEG = -30000.0               # fp16-safe; SCALE*MASK_NEG << -90 => exp = 0
XSH = D_MODEL // N_CORES          # x shard rows per core = 768


def _chunk_plan(kd, chunk):
    """Chunk sizes: small leading chunks so the PE starts early."""
    plan = []
    rest = kd
    if kd > 8:
        plan = [1, 2, 3]
        rest = kd - 6
    while rest > 0:
        c = min(chunk, rest)
        plan.append(c)
        rest -= c
    return plan


def build_program(d_model=D_MODEL, t=T, nqh=NQH, chunk=6, n_cores=N_CORES,
                  use_cc=True):
    """Build the per-core Bass program. Returns the compiled Bacc handle.

    use_cc=False builds a single-core variant (full xT input, full partial
    output, no collectives) for CoreSim numerical checking.
    """
    kd = d_model // 128            # contraction tiles for qkv projection
    tq = t // 512                  # 512-wide T quads
    tb = t // 128                  # 128-wide T blocks
    nf = nqh + 2                   # feature tiles: q heads + k + v
    qkv_cols = nf * 128
    xsh = d_model // n_cores       # x shard rows

    nc = bacc.Bacc("TRN2", target_bir_lowering=False, debug=False,
                   num_devices=n_cores)

    if use_cc:
        xs = nc.dram_tensor("xs", [xsh, t], F16, kind="ExternalInput").ap()
    else:
        xs = nc.dram_tensor("xs", [d_model, t], F16, kind="ExternalInput").ap()
    wqkvT = nc.dram_tensor("wqkvT", [d_model, qkv_cols], F16,
                           kind="ExternalInput").ap()
    w_o = nc.dram_tensor("w_o", [nqh * 128, d_model], F16,
                         kind="ExternalInput").ap()
    cosf = nc.dram_tensor("cosf", [128, t], F16, kind="ExternalInput").ap()
    sinf = nc.dram_tensor("sinf", [128, t], F16, kind="ExternalInput").ap()
    maskT = nc.dram_tensor("maskT", [4, 128, 512], F16, kind="ExternalInput").ap()
    pswap = nc.dram_tensor("pswap", [128, 128], F32, kind="ExternalInput").ap()
    ident = nc.dram_tensor("ident", [128, 128], F32, kind="ExternalInput").ap()
    ones = nc.dram_tensor("ones", [128, 1], F32, kind="ExternalInput").ap()
    if use_cc:
        outT = nc.dram_tensor("outT", [xsh, t], F16, kind="ExternalOutput").ap()
    else:
        outT = nc.dram_tensor("outT", [d_model, t], F16, kind="ExternalOutput").ap()

    def _body(tc):
        with ExitStack() as octx:
            dram = octx.enter_context(tc.tile_pool(name="dram", bufs=1,
                                                   space="DRAM"))
            if use_cc:
                # all-gather the x shard into a full on-device xT
                xg_in = dram.tile([xsh, t], F16, tag="xg_in")
                nc.gpsimd.dma_start(xg_in[:], xs[:])
                xT = dram.tile([d_model, t], F16, tag="xg")
                nc.gpsimd.collective_compute(
                    "AllGather", mybir.AluOpType.bypass,
                    replica_groups=[list(range(n_cores))],
                    ins=[xg_in.opt()], outs=[xT.opt()])
                xT = xT[:]
            else:
                xT = xs

            consts = octx.enter_context(tc.tile_pool(name="consts", bufs=1))
            accp = octx.enter_context(tc.tile_pool(name="accp", bufs=nf))

            # fp16 staging + on-chip convert of the DVE-facing constants
            stg_cos = consts.tile([128, t], F16, tag="stg_cos")
            nc.sync.dma_start(stg_cos[:], cosf[:])
            stg_sin = consts.tile([128, t], F16, tag="stg_sin")
            nc.sync.dma_start(stg_sin[:], sinf[:])
            stg_mask = consts.tile([128, 4, 512], F16, tag="stg_mask")
            nc.sync.dma_start(stg_mask[:], maskT.transpose([1, 0, 2]))

            sb_cos = consts.tile([128, t], F32, tag="cos")
            nc.vector.tensor_copy(sb_cos[:], stg_cos[:])
            sb_sin = consts.tile([128, t], F32, tag="sin")
            nc.vector.tensor_copy(sb_sin[:], stg_sin[:])
            sb_mask = consts.tile([128, 4, 512], F32, tag="mask")
            nc.vector.tensor_copy(sb_mask[:], stg_mask[:])
            sb_pswap = consts.tile([128, 128], F32R, tag="pswap")
            nc.gpsimd.dma_start(sb_pswap[:], pswap[:])
            sb_ident = consts.tile([128, 128], F32R, tag="ident")
            nc.gpsimd.dma_start(sb_ident[:], ident[:])
            sb_ones = consts.tile([128, 1], F32R, tag="ones")
            nc.gpsimd.dma_start(sb_ones[:], ones[:])

            acc = [accp.tile([128, t], F32R, tag="acc", name=f"acc{i}")
                   for i in range(nf)]

            # ---------------- Phase 1: qkvT = wqkvT-tiles.T @ xT ----------------
            # fp16 matmuls; weight-stationary: one w block feeds 4 PSUM banks.
            with ExitStack() as ctx:
                xp = ctx.enter_context(tc.tile_pool(name="xp", bufs=10))
                wp = ctx.enter_context(tc.tile_pool(name="wp", bufs=9))
                psp = ctx.enter_context(tc.tile_pool(name="psp", bufs=2 * tq,
                                                     space="PSUM"))
                plan = _chunk_plan(kd, chunk)
                base = 0
                for c, csz in enumerate(plan):
                    ks = list(range(base, base + csz))
                    base += csz
                    xts, wts = [], []
                    for k in ks:
                        xt = xp.tile([128, t], F16, tag="xt")
                        nc.gpsimd.dma_start(xt[:], xT[k * 128:(k + 1) * 128, :])
                        xts.append(xt)
                        wt = wp.tile([128, qkv_cols], F16, tag="wt")
                        nc.gpsimd.dma_start(wt[:], wqkvT[k * 128:(k + 1) * 128, :])
                        wts.append(wt)
                    for f in range(nf):
                        pss = [psp.tile([128, 512], F32, tag="ps", name=f"ps{q}")
                               for q in range(tq)]
                        for i in range(len(ks)):
                            lhs = wts[i][:, f * 128:(f + 1) * 128]
                            for q in range(tq):
                                nc.tensor.matmul(
                                    pss[q][:], lhs, xts[i][:, q * 512:(q + 1) * 512],
                                    start=(i == 0), stop=(i == len(ks) - 1),
                                )
                        for q in range(tq):
                            sl = slice(q * 512, (q + 1) * 512)
                            if c == 0:
                                nc.vector.tensor_copy(acc[f][:, sl], pss[q][:])
                            else:
                                nc.vector.scalar_tensor_tensor(
                                    out=acc[f][:, sl], in0=pss[q][:], scalar=1.0,
                                    in1=acc[f][:, sl], op0=MULT, op1=ADD,
                                )

            # late pools: open only after phase 1 frees its SBUF
            attp = octx.enter_context(tc.tile_pool(name="attp", bufs=nqh))
            vtp = octx.enter_context(tc.tile_pool(name="vtp", bufs=1))
            attnT = [attp.tile([128, t], F16, tag="attnT", name=f"attnT{i}")
                     for i in range(nqh)]
            v_t = vtp.tile([128, tb, 128], F32R, tag="v_t")

            # ---------------- Phase 1.5: RoPE + clamp + v transpose ----------------
            # Emission order matters (DVE is FIFO): k head first, then v work,
            # then q heads in pair order — the attention loop below is
            # hp-outer, so rope for later head pairs hides under the PE work
            # of earlier pairs.
            rope_ctx = ExitStack()
            # shared PSUM pool: rope pair-swap tiles + attention score tiles
            spp = rope_ctx.enter_context(tc.tile_pool(name="spp", bufs=4,
                                                      space="PSUM"))
            tmp = rope_ctx.enter_context(tc.tile_pool(name="rtmp", bufs=3))

            def emit_rope(f):
                for q in range(tq):
                    sl = slice(q * 512, (q + 1) * 512)
                    pshuf = spp.tile([128, 512], F32, tag="ps", name="pshuf")
                    nc.tensor.matmul(pshuf[:], sb_pswap[:], acc[f][:, sl],
                                     start=True, stop=True)
                    t1 = tmp.tile([128, 512], F32, tag="t1", name="t1")
                    nc.vector.tensor_tensor(t1[:], acc[f][:, sl].bitcast(F32),
                                            sb_cos[:, sl], op=MULT)
                    t2 = tmp.tile([128, 512], F32, tag="t2", name="t2")
                    nc.vector.tensor_tensor(t2[:], pshuf[:], sb_sin[:, sl], op=MULT)
                    t3 = tmp.tile([128, 512], F32, tag="t3", name="t3")
                    nc.vector.tensor_tensor(t3[:], t1[:], t2[:], op=ADD)
                    nc.vector.tensor_scalar(acc[f][:, sl], t3[:], -CLAMP, CLAMP,
                                            op0=MAX, op1=MIN)

            def emit_vwork():
                iv = nqh + 1                      # v: clamp only, then transpose
                for q in range(tq):
                    sl = slice(q * 512, (q + 1) * 512)
                    t4 = tmp.tile([128, 512], F32, tag="t1", name="t1")
                    nc.vector.tensor_scalar(t4[:], acc[iv][:, sl].bitcast(F32),
                                            -CLAMP, CLAMP, op0=MAX, op1=MIN)
                    nc.vector.tensor_copy(acc[iv][:, sl], t4[:])
                for j in range(tb):
                    pt = spp.tile([128, 128], F32R, tag="ps", name="pt")
                    nc.tensor.transpose(pt[:], acc[iv][:, j * 128:(j + 1) * 128],
                                        sb_ident[:])
                    nc.vector.tensor_copy(v_t[:, j, :], pt[:])

            emit_rope(nqh)                        # k head
            emit_vwork()

            # ---------------- Phase 2: causal attention (head pairs) ----------------
            # Software-pipelined: score matmuls + exp run PIPE kb-steps ahead
            # of the PV/denominator matmuls so the PE never waits on the ACT
            # round-trip. PSUM: 4 score banks + 2 attn banks + 2 sum banks.
            # hp-outer so emit_rope(f) for pair p+1 overlaps pair p's matmuls.
            PIPE = 2
            with ExitStack() as ctx:
                exps = ctx.enter_context(tc.tile_pool(name="exps", bufs=2 * (PIPE + 1)))
                psa_p = ctx.enter_context(tc.tile_pool(name="psa", bufs=2, space="PSUM"))
                psm_p = ctx.enter_context(tc.tile_pool(name="psm", bufs=2, space="PSUM"))
                rcp = ctx.enter_context(tc.tile_pool(name="rcp", bufs=2))
                rcb = ctx.enter_context(tc.tile_pool(name="rcb", bufs=2))
                ik = nqh                          # k head feature tile
                for hp in range(nqh // 2):
                    hs = (2 * hp, 2 * hp + 1)
                    emit_rope(hs[0])
                    emit_rope(hs[1])
                    for J in range(tq):
                        qsl = slice(J * 512, (J + 1) * 512)
                        nkb = 4 * J + 4
                        pa = {h: psa_p.tile([128, 512], F32, tag="pa", name="pa")
                              for h in hs}
                        pm = {h: psm_p.tile([1, 512], F32, tag="pm", name="pm")
                              for h in hs}
                        exq = {}
                        for kb in range(nkb + PIPE):
                            if kb < nkb:
                                klhs = acc[ik][:, kb * 128:(kb + 1) * 128]
                                for h in hs:
                                    ps = spp.tile([128, 512], F32, tag="ps",
                                                  name="ps")
                                    nc.tensor.matmul(ps[:], klhs, acc[h][:, qsl],
                                                     start=True, stop=True)
                                    if kb >= 4 * J:
                                        nc.vector.tensor_tensor(
                                            ps[:], ps[:],
                                            sb_mask[:, kb - 4 * J, :], op=ADD)
                                    ex = exps.tile([128, 512], F32R, tag="ex",
                                                   name="ex")
                                    nc.scalar.activation(ex[:], ps[:], EXP,
                                                         scale=SCALE)
                                    exq[(kb, h)] = ex
                            kbd = kb - PIPE
                            if kbd < 0 or kbd >= nkb:
                                continue
                            st = (kbd == 0)
                            sp = (kbd == nkb - 1)
                            for h in hs:
                                nc.tensor.matmul(pa[h][:], v_t[:, kbd, :],
                                                 exq[(kbd, h)][:], start=st, stop=sp)
                            for h in hs:
                                nc.tensor.matmul(pm[h][:], sb_ones[:],
                                                 exq.pop((kbd, h))[:],
                                                 start=st, stop=sp)
                        for h in hs:
                            rc = rcp.tile([1, 512], F32R, tag="rc", name="rc")
                            nc.vector.reciprocal(rc[:], pm[h][:])
                            rb = rcb.tile([128, 512], F32R, tag="rb", name="rb")
                            nc.sync.dma_start(
                                out=rb[:],
                                in_=bass.AP(tensor=rc.tensor, offset=rc.offset,
                                            ap=[[1, 1], [0, 128]] + rc.ap[1:]))
                            nc.vector.tensor_tensor(attnT[h][:, qsl], pa[h][:],
                                                    rb[:], op=MULT)

            rope_ctx.close()

            # ---------------- Phase 3: partial out projection (fp16) ----------------
            # Weight-stationary: one w_o block feeds all 4 t-quads. Partials
            # land in a DRAM buffer, then on-device ReduceScatter sums them
            # and leaves this core's [xsh, t] shard for the tiny output DMA.
            if use_cc:
                part = dram.tile([d_model, t], F16, tag="part")
                rs_out = dram.tile([xsh, t], F16, tag="rs_out")
            with ExitStack() as ctx:
                wop = ctx.enter_context(tc.tile_pool(name="wop", bufs=4))
                outp = ctx.enter_context(tc.tile_pool(name="outp", bufs=2 * tq))
                pso = ctx.enter_context(tc.tile_pool(name="pso", bufs=2 * tq,
                                                     space="PSUM"))
                for o in range(d_model // 128):
                    wo = wop.tile([128, nqh, 128], F16, tag="wo")
                    src = w_o[:, o * 128:(o + 1) * 128].rearrange(
                        "(s p) o -> p s o", p=128)
                    nc.gpsimd.dma_start(wo[:], src)
                    pos = [pso.tile([128, 512], F32, tag="po", name=f"po{q}")
                           for q in range(tq)]
                    for s in range(nqh):
                        lhs = wo[:, s, :]
                        for J in range(tq):
                            nc.tensor.matmul(pos[J][:], lhs,
                                             attnT[s][:, J * 512:(J + 1) * 512],
                                             start=(s == 0), stop=(s == nqh - 1))
                    for J in range(tq):
                        ob = outp.tile([128, 512], F16, tag="ob", name="ob")
                        nc.vector.tensor_copy(ob[:], pos[J][:])
                        dst = part if use_cc else outT
                        nc.sync.dma_start(
                            dst[o * 128:(o + 1) * 128, J * 512:(J + 1) * 512],
                            ob[:])

            if use_cc:
                nc.gpsimd.collective_compute(
                    "ReduceScatter", ADD,
                    replica_groups=[list(range(n_cores))],
                    ins=[part.opt()], outs=[rs_out.opt()])
                nc.gpsimd.dma_start(outT[:], rs_out[:])

    with tile.TileContext(nc) as tc, nc.allow_low_precision(reason="fp16/fp32r matmuls"):
        _body(tc)
    nc.compile()
    return nc


def make_core_inputs(x, causal_mask, w_qkv, w_out, d_model=D_MODEL, t=T, nqh=NQH,
                     n_cores=N_CORES, use_cc=True):
    """Host-side sharding: per-core input dicts for the SPMD program."""
    x2 = np.ascontiguousarray(x.reshape(t, d_model).T).astype(np.float16)  # [D, T]
    kv_base = nqh * n_cores * HEAD_DIM
    n_kv = n_cores
    xsh = d_model // n_cores

    # RoPE tables in [head_dim, T] layout (interleaved-pair convention).
    hd = HEAD_DIM
    inv = 1.0 / ROPE_BASE ** (np.arange(0, hd, 2, dtype=np.float64) / hd)
    pos = np.arange(t, dtype=np.float64)
    freqs = pos[None, :] * inv[:, None]                     # [hd/2, T]
    cos = np.cos(freqs)
    sin = np.sin(freqs)
    cosf = np.empty((hd, t), dtype=np.float16)
    sinf = np.empty((hd, t), dtype=np.float16)
    cosf[0::2] = cos
    cosf[1::2] = cos
    sinf[0::2] = -sin                                       # row 2i:   -sin
    sinf[1::2] = sin                                        # row 2i+1: +sin

    # causal mask diagonal-quad slices, clipped to an fp16-safe big-negative
    cm = causal_mask.reshape(causal_mask.shape[-2], causal_mask.shape[-1])
    cm = np.maximum(np.asarray(cm, dtype=np.float32), MASK_NEG)
    maskT = np.stack([np.ascontiguousarray(cm[0:512, m * 128:(m + 1) * 128].T)
                      for m in range(4)]).astype(np.float16)

    pswap = np.zeros((128, 128), dtype=np.float32)
    for i in range(0, 128, 2):
        pswap[i, i + 1] = 1.0
        pswap[i + 1, i] = 1.0
    ident = np.eye(128, dtype=np.float32)

    in_maps = []
    for c in range(n_cores):
        qrows = np.arange(c * nqh * 128, (c + 1) * nqh * 128)
        krows = np.arange(kv_base + c * 128, kv_base + (c + 1) * 128)
        vrows = np.arange(kv_base + n_kv * 128 + c * 128,
                          kv_base + n_kv * 128 + (c + 1) * 128)
        rows = np.concatenate([qrows, krows, vrows])
        wqkvT_c = np.ascontiguousarray(w_qkv[rows, :].T).astype(np.float16)
        w_o_c = np.ascontiguousarray(w_out[:, qrows].T).astype(np.float16)
        xs_c = (np.ascontiguousarray(x2[c * xsh:(c + 1) * xsh, :])
                if use_cc else x2)
        in_maps.append({
            "xs": xs_c, "wqkvT": wqkvT_c, "w_o": w_o_c,
            "cosf": cosf, "sinf": sinf, "maskT": maskT,
            "pswap": pswap, "ident": ident,
            "ones": np.ones((128, 1), dtype=np.float32),
        })
    return in_maps


_PROGRAM_CACHE = {}


def _get_program():
    key = (D_MODEL, T, NQH)
    if key not in _PROGRAM_CACHE:
        _PROGRAM_CACHE[key] = build_program()
    return _PROGRAM_CACHE[key]


def kernel(x, causal_mask, w_qkv, w_out):
    x = np.asarray(x, dtype=np.float32)
    causal_mask = np.asarray(causal_mask, dtype=np.float32)
    w_qkv = np.asarray(w_qkv, dtype=np.float32)
    w_out = np.asarray(w_out, dtype=np.float32)

    nc = _get_program()
    in_maps = make_core_inputs(x, causal_mask, w_qkv, w_out)
    res = run_bass_kernel_spmd(nc, in_maps, list(range(N_CORES)))
    shards = [np.asarray(res.results[c]["outT"], dtype=np.float32)
              for c in range(N_CORES)]
    outT = np.concatenate(shards, axis=0)                   # [D, T]
    return np.ascontiguousarray(outT.T).reshape(1, T, D_MODEL).astype(np.float32)
